# revision 14
# baseline (speedup 1.0000x reference)
"""Trainium2 Bass kernel for nn_ExperimentalLoss_23742579212660.

Loss = mean(0.2*G + 0.8*mse) where
  mse  = masked MSE over valid (target > 0) pixels,
  G    = blur3x3+sobel3x3(target) - blur3x3+sobel3x3(pred)  (reflect-101 pads).

Algebraic structure exploited:
  * mean(0.2*G + 0.8*mse) = 0.2*mean(G) + 0.8*mse.
  * The two stacked reflect-101 3x3 convs equal ONE separable 5-tap conv with
    c = [-1,-2,0,2,1]/4 per axis; sum(c)=0 makes the interior weight of
    sum(G) vanish, so mean(G) collapses to a fixed 36-term weighted sum of
    (target - pred) corner pixels, computed exactly on host from the f32
    inputs (~1e-8 here).
  * The memory-bound part is the masked MSE. The explicit 2e-2 error budget
    admits reduced input precision: inputs are rounded (RTNE) to fp8-e4m3 on
    host, quartering HBM traffic. Measured end-to-end effect on this input
    distribution: ~1.6e-5 relative (vs f32).
  * Row-block sharded over 8 NeuronCores; each core streams its [512, 4096]
    slice (relaid out as [128, 16384]) of both tensors and emits [128, NC]
    column partials; host reduces in f64.

Device: the column range is split into jobs consumed by two parallel pipes
(DVE is the throughput limit at 1 elem/cycle/lane for fp8, so part of the
work is rerouted to the otherwise-idle GPSIMD+ACT engines):
  'D' jobs -> DVE custom fused op  out = (t - p*(t>0))^2, accum -> sq col
              ( == mask*(t-p)^2 exactly, since t*mask == t )
  'G' jobs -> polarization sum (t-p)^2 = t^2 - 2tp + p^2:
              GPSIMD scalar_tensor_tensor (t*1)*p, accum -> tp col
              ACT    Square(t) accum -> t2 col, Square(p) accum -> p2 col
              (masking dropped here: it only differs where t == 0 exactly,
               a ~1e-7 relative effect on this input distribution)
  count  -> host popcount of nonzero fp8 bytes (t >= 0, so t > 0 iff the
            fp8 byte is nonzero) — bit-identical to the device Sign-accum
            it replaces, and off the device critical path.
DMA: three parallel descriptor-generation queues (Sync HWDGE, Scalar HWDGE,
GPSIMD SWDGE), with t_i and p_i ADJACENT in the same queue so FIFO order
guarantees each pair completes back-to-back. All tiles are SBUF-resident
(no buffer recycling). One combined [128, NC] f32 result DMA at the end.
"""

import sys

import numpy as np

for _p in ("/opt/trn_rl_repo",):
    if _p not in sys.path:
        sys.path.insert(0, _p)

import ml_dtypes

H = 4096
W = 4096
N_CORES = 8
ROWS_PER_CORE = H // N_CORES          # 512
P = 128                               # SBUF partitions
COLS = ROWS_PER_CORE * W // P         # 16384 (per-core data as [128, 16384])

# (width, ring, class): ring 0=Sync HWDGE, 1=Scalar HWDGE, 2=GPSIMD SWDGE.
# Widths ramp up so the DVE/GPSIMD pipes start as soon as possible; the
# per-queue descriptor-generation (~0.65us/DMA, 4-deep window) paces early
# delivery, so tiny leading tiles would not arrive any sooner.
JOBS = (
    (512,  0, "D"),
    (1024, 1, "D"),
    (1536, 1, "D"),
    (1536, 2, "G"),
    (1536, 2, "G"),
    (2560, 0, "D"),
    (3584, 0, "D"),
    (4096, 1, "D"),
)
assert sum(w for w, _, _ in JOBS) == COLS
N_D = sum(1 for _, _, c in JOBS if c == "D")
N_G = sum(1 for _, _, c in JOBS if c == "G")
NACC = N_D + 3 * N_G                  # acc cols: sq per D; tp,t2,p2 per G

HOST_DT = ml_dtypes.float8_e4m3       # matches device float8e4 decode

# Per-axis boundary weights of sum(G) (antisymmetric; interior weight is 0).
_BORDER_IDX = (0, 1, 2, H - 3, H - 2, H - 1)
_BORDER_W = (-0.75, -1.0, -0.25, 0.25, 1.0, 0.75)

_CACHED_NC = None


def _register_custom_op(name, spec):
    """Register a custom DVE op at runtime. The micro-op table is generated
    per-NEFF, so no firmware change is involved -- same mechanism as the
    production dve_ops.OPS entries."""
    import concourse.dve_ops as dve_ops
    from concourse.dve_spec import lower, _has_src1
    from concourse.dve_uop import DveOpSpec
    from concourse.dve_table_gen import dve_ver_for

    for op in dve_ops.OPS:
        if op.name == name:
            return op
    op = dve_ops.DveOp(name, spec, subdim=False, uops_sha={})
    dve_ops.OPS.append(op)
    dve_ops.CUSTOM_DVE_SPECS[name] = spec
    dve_ops._SUB_OPCODE_FOR_NAME[name] = (
        dve_ops._CUSTOM_DVE_ROW_BASE + len(dve_ops.OPS) - 1
    )
    ver = dve_ver_for("TRN2")
    dve_ops._COMPILE_CACHE[(name, ver)] = DveOpSpec(
        name=name,
        opcode=dve_ops.get_dve_sub_opcode(name),
        uops=lower(spec, ver=ver),
        rd1_en=_has_src1(spec),
    )
    return op


def _masked_sqdiff_op():
    """Fused DVE op: out = (in0 - in1*(in0>0))^2, accum_out = s0 + sum(out)."""
    from concourse.dve_spec import Spec, Src0, Src1, Zero, sq, C0
    from operator import add

    def _ref(in0, in1, s0, s1, imm2):
        m = (in0 > 0).astype(np.float32)
        b = ((in0.astype(np.float32) - in1 * m) ** 2).astype(np.float32)
        return b, s0 + b.reshape(b.shape[0], -1).sum(axis=-1, keepdims=True)

    return _register_custom_op(
        "MASKED_SQDIFF_LOSS_ANT",
        Spec(body=sq(Src0 - Src1 * (Src0 > Zero)), accum=add, accum_init=C0,
             reference=_ref),
    )


def _build_program():
    global _CACHED_NC
    if _CACHED_NC is not None:
        return _CACHED_NC

    from concourse import bacc, mybir
    import concourse.tile as tile

    f32 = mybir.dt.float32
    f8 = mybir.dt.float8e4
    AF = mybir.ActivationFunctionType
    ALU = mybir.AluOpType
    msd_op = _masked_sqdiff_op()

    nc = bacc.Bacc(
        "TRN2",
        debug=False,
        target_bir_lowering=False,
        num_devices=N_CORES,
        enable_partition_id=False,
        enable_asserts=False,
    )
    t_d = nc.dram_tensor("t", [P, COLS], f8, kind="ExternalInput").ap()
    p_d = nc.dram_tensor("p", [P, COLS], f8, kind="ExternalInput").ap()
    out_d = nc.dram_tensor("o", [P, NACC], f32, kind="ExternalOutput").ap()

    max_d = max(w for w, _, c in JOBS if c == "D")
    max_g = max(w for w, _, c in JOBS if c == "G")

    with tile.TileContext(nc) as tc:
        with (
            tc.tile_pool(name="tin", bufs=1) as tpool,
            tc.tile_pool(name="pin", bufs=1) as ppool,
            tc.tile_pool(name="dsq", bufs=2) as qpool,
            tc.tile_pool(name="gq", bufs=2) as gpool,
            tc.tile_pool(name="asq", bufs=2) as mpool,
            tc.tile_pool(name="acc", bufs=1) as apool,
        ):
            acc = apool.tile([P, NACC], f32, tag="acc")
            rings = {0: nc.sync, 1: nc.scalar, 2: nc.gpsimd}

            tts, pts = [], []
            col = 0
            for i, (w, ring, _) in enumerate(JOBS):
                cs = slice(col, col + w)
                col += w
                tt = tpool.tile([P, w], f8, tag=f"t{i}", bufs=1)
                rings[ring].dma_start(out=tt[:], in_=t_d[:, cs])
                tts.append(tt)
                pt = ppool.tile([P, w], f8, tag=f"p{i}", bufs=1)
                rings[ring].dma_start(out=pt[:], in_=p_d[:, cs])
                pts.append(pt)

            # Pool has no fused multiply-accumulate, so each G job runs:
            #   GPSIMD: prod = t * p          (fp8 in, f32 out)
            #   ACT   : Square(t), Square(p)  (accum -> t2, p2 cols)
            #   ACT   : Identity(prod)        (accum -> tp col; emitted after
            #           all Squares so it never head-of-line blocks them)
            ci = 0
            g_reduce = []
            for i, (w, _, cl) in enumerate(JOBS):
                if cl == "D":
                    dsq = qpool.tile([P, w], f8, tag="q", padded_shape=[P, max_d])
                    nc.vector._custom_dve(
                        msd_op,
                        out=dsq[:], in0=tts[i][:], in1=pts[i][:],
                        s0=0.0, s1=0.0,
                        accum_out=acc[:, ci : ci + 1],
                    )
                    ci += 1
                else:
                    gq = gpool.tile([P, w], f32, tag="g", padded_shape=[P, max_g])
                    nc.gpsimd.tensor_tensor(
                        out=gq[:], in0=tts[i][:], in1=pts[i][:], op=ALU.mult,
                    )
                    at = mpool.tile([P, w], f8, tag="a", padded_shape=[P, max_g])
                    nc.scalar.activation(
                        out=at[:], in_=tts[i][:], func=AF.Square,
                        accum_out=acc[:, ci + 1 : ci + 2],
                    )
                    ap_ = mpool.tile([P, w], f8, tag="a", padded_shape=[P, max_g])
                    nc.scalar.activation(
                        out=ap_[:], in_=pts[i][:], func=AF.Square,
                        accum_out=acc[:, ci + 2 : ci + 3],
                    )
                    g_reduce.append((gq, ci))
                    ci += 3

            for gq, gci in g_reduce:
                nc.scalar.activation(
                    out=gq[:], in_=gq[:], func=AF.Identity,
                    accum_out=acc[:, gci : gci + 1],
                )

            nc.sync.dma_start(out=out_d[:], in_=acc[:])

    nc.compile()
    _CACHED_NC = nc
    return nc


def _pack_cores(t2: np.ndarray, p2: np.ndarray):
    """Round both images to fp8 (RTNE) and lay each core's row block out as
    [128, 16384] (any bijective relayout is valid: the device only reduces)."""
    t8 = t2.astype(HOST_DT)
    p8 = p2.astype(HOST_DT)
    in_maps = []
    for c in range(N_CORES):
        rs = slice(c * ROWS_PER_CORE, (c + 1) * ROWS_PER_CORE)
        in_maps.append({
            "t": np.ascontiguousarray(t8[rs]).reshape(P, COLS),
            "p": np.ascontiguousarray(p8[rs]).reshape(P, COLS),
        })
    return in_maps, t8


def _run_device(t2: np.ndarray, p2: np.ndarray, trace: bool = False):
    from concourse.bass_utils import run_bass_kernel_spmd

    nc = _build_program()
    in_maps, _ = _pack_cores(t2, p2)
    return run_bass_kernel_spmd(nc, in_maps, list(range(N_CORES)), trace=trace)


def kernel(pred: np.ndarray, target: np.ndarray) -> np.ndarray:
    p2 = np.ascontiguousarray(np.asarray(pred, dtype=np.float32).reshape(H, W))
    t2 = np.ascontiguousarray(np.asarray(target, dtype=np.float32).reshape(H, W))

    from concourse.bass_utils import run_bass_kernel_spmd

    nc = _build_program()
    in_maps, t8 = _pack_cores(t2, p2)
    results = run_bass_kernel_spmd(nc, in_maps, list(range(N_CORES))).results

    # count(t > 0): t >= 0, so the fp8 byte is nonzero iff t > 0.  This is
    # bit-identical to accumulating Sign(t) over the same fp8 tensor.
    count = float(np.count_nonzero(t8.view(np.uint8)))

    S = 0.0
    for c in range(N_CORES):
        o = results[c]["o"].astype(np.float64)
        ci = 0
        for w, _, cl in JOBS:
            if cl == "D":
                S += float(o[:, ci].sum())
                ci += 1
            else:
                tp = float(o[:, ci].sum())
                tsq = float(o[:, ci + 1].sum())
                psq = float(o[:, ci + 2].sum())
                S += tsq - 2.0 * tp + psq
                ci += 3
    mse = S / max(count, 1.0)

    corner = 0.0
    for wi, i in zip(_BORDER_W, _BORDER_IDX):
        for wj, j in zip(_BORDER_W, _BORDER_IDX):
            corner += wi * wj * (float(t2[i, j]) - float(p2[i, j]))
    mean_g = corner / (H * W)

    return np.asarray(0.2 * mean_g + 0.8 * mse, dtype=np.float32)


# revision 15
# speedup vs baseline: 1.0283x; 1.0283x over previous
"""Trainium2 Bass kernel for nn_ExperimentalLoss_23742579212660.

Loss = mean(0.2*G + 0.8*mse) where
  mse  = masked MSE over valid (target > 0) pixels,
  G    = blur3x3+sobel3x3(target) - blur3x3+sobel3x3(pred)  (reflect-101 pads).

Algebraic structure exploited:
  * mean(0.2*G + 0.8*mse) = 0.2*mean(G) + 0.8*mse.
  * The two stacked reflect-101 3x3 convs equal ONE separable 5-tap conv with
    c = [-1,-2,0,2,1]/4 per axis; sum(c)=0 makes the interior weight of
    sum(G) vanish, so mean(G) collapses to a fixed 36-term weighted sum of
    (target - pred) corner pixels, computed exactly on host from the f32
    inputs (~1e-8 here).
  * The memory-bound part is the masked MSE. The explicit 2e-2 error budget
    admits reduced input precision: inputs are rounded (RTNE) to fp8-e4m3 on
    host, quartering HBM traffic. Measured end-to-end effect on this input
    distribution: ~1.6e-5 relative (vs f32).
  * Row-block sharded over 8 NeuronCores; each core streams its [512, 4096]
    slice (relaid out as [128, 16384]) of both tensors and emits [128, 2*NJ]
    column partials of sum(mask*(t-p)^2) and sum(mask); host reduces in f64.

Device per tile [128, w] (one pass per engine, all tiles SBUF-resident):
  DVE : custom fused op  out = (t - p*(t>0))^2, accum -> sq col
        ( == mask*(t-p)^2 exactly, since t*mask == t )
  ACT : mask = Sign(t)   (t >= 0, so Sign == (t > 0)), accum -> count col
DVE at 1 elem/cycle/lane (fp8 has no packed DVE mode, and custom-op tables
only carry the 1x program) is the throughput limit; offloading to GPSIMD was
measured to HALVE overlapped DVE ops (shared SBUF ports knock DVE out of its
2-port mode), so everything elementwise stays on DVE/ACT.

DMA: pair i rides HWDGE ring i%2 (Sync/Scalar) with t_i and p_i ADJACENT in
the same queue, so FIFO order guarantees each pair completes back-to-back
(t/p on separate rings skews pairs by several us via coarse per-queue packet
round-robin; a single ring for everything is paced by its ~0.63us/DMA
descriptor generation and 4-deep gen window).  All tiles are SBUF-resident
(no buffer recycling), so rings never stall on buffer-release semaphores.
One combined [128, 2*NJ] f32 result DMA at the end.
"""

import sys

import numpy as np

for _p in ("/opt/trn_rl_repo",):
    if _p not in sys.path:
        sys.path.insert(0, _p)

import ml_dtypes

H = 4096
W = 4096
N_CORES = 8
ROWS_PER_CORE = H // N_CORES          # 512
P = 128                               # SBUF partitions
COLS = ROWS_PER_CORE * W // P         # 16384 (per-core data as [128, 16384])
JOB_COLS = (1536, 2048, 2560, 3072, 3584, 3584)
assert sum(JOB_COLS) == COLS
NJ = len(JOB_COLS)

HOST_DT = ml_dtypes.float8_e4m3       # matches device float8e4 decode

# Per-axis boundary weights of sum(G) (antisymmetric; interior weight is 0).
_BORDER_IDX = (0, 1, 2, H - 3, H - 2, H - 1)
_BORDER_W = (-0.75, -1.0, -0.25, 0.25, 1.0, 0.75)

_CACHED_NC = None


def _register_custom_op(name, spec):
    """Register a custom DVE op at runtime. The micro-op table is generated
    per-NEFF, so no firmware change is involved -- same mechanism as the
    production dve_ops.OPS entries."""
    import concourse.dve_ops as dve_ops
    from concourse.dve_spec import lower, _has_src1
    from concourse.dve_uop import DveOpSpec
    from concourse.dve_table_gen import dve_ver_for

    for op in dve_ops.OPS:
        if op.name == name:
            return op
    op = dve_ops.DveOp(name, spec, subdim=False, uops_sha={})
    dve_ops.OPS.append(op)
    dve_ops.CUSTOM_DVE_SPECS[name] = spec
    dve_ops._SUB_OPCODE_FOR_NAME[name] = (
        dve_ops._CUSTOM_DVE_ROW_BASE + len(dve_ops.OPS) - 1
    )
    ver = dve_ver_for("TRN2")
    dve_ops._COMPILE_CACHE[(name, ver)] = DveOpSpec(
        name=name,
        opcode=dve_ops.get_dve_sub_opcode(name),
        uops=lower(spec, ver=ver),
        rd1_en=_has_src1(spec),
    )
    return op


def _masked_sqdiff_op():
    """Fused DVE op: out = (in0 - in1*(in0>0))^2, accum_out = s0 + sum(out)."""
    from concourse.dve_spec import Spec, Src0, Src1, Zero, sq, C0
    from operator import add

    def _ref(in0, in1, s0, s1, imm2):
        m = (in0 > 0).astype(np.float32)
        b = ((in0.astype(np.float32) - in1 * m) ** 2).astype(np.float32)
        return b, s0 + b.reshape(b.shape[0], -1).sum(axis=-1, keepdims=True)

    return _register_custom_op(
        "MASKED_SQDIFF_LOSS_ANT",
        Spec(body=sq(Src0 - Src1 * (Src0 > Zero)), accum=add, accum_init=C0,
             reference=_ref),
    )


def _build_program():
    global _CACHED_NC
    if _CACHED_NC is not None:
        return _CACHED_NC

    from concourse import bacc, mybir
    import concourse.tile as tile

    f32 = mybir.dt.float32
    f8 = mybir.dt.float8e4
    AF = mybir.ActivationFunctionType
    msd_op = _masked_sqdiff_op()

    nc = bacc.Bacc(
        "TRN2",
        debug=False,
        target_bir_lowering=False,
        num_devices=N_CORES,
        enable_partition_id=False,
        enable_asserts=False,
    )
    t_d = nc.dram_tensor("t", [P, COLS], f8, kind="ExternalInput").ap()
    p_d = nc.dram_tensor("p", [P, COLS], f8, kind="ExternalInput").ap()
    out_d = nc.dram_tensor("o", [P, 2 * NJ], f32, kind="ExternalOutput").ap()

    col0 = [sum(JOB_COLS[:i]) for i in range(NJ)]
    max_w = max(JOB_COLS)

    with tile.TileContext(nc) as tc:
        with (
            tc.tile_pool(name="tin", bufs=1) as tpool,
            tc.tile_pool(name="pin", bufs=1) as ppool,
            tc.tile_pool(name="mask", bufs=2) as mpool,
            tc.tile_pool(name="dsq", bufs=2) as qpool,
            tc.tile_pool(name="acc", bufs=1) as apool,
        ):
            acc = apool.tile([P, 2 * NJ], f32, tag="acc")

            tts, pts = [], []
            for i, w in enumerate(JOB_COLS):
                cs = slice(col0[i], col0[i] + w)
                ring = nc.sync if i % 2 == 0 else nc.scalar
                tt = tpool.tile([P, w], f8, tag=f"t{i}", bufs=1)
                ring.dma_start(out=tt[:], in_=t_d[:, cs])
                tts.append(tt)
                pt = ppool.tile([P, w], f8, tag=f"p{i}", bufs=1)
                ring.dma_start(out=pt[:], in_=p_d[:, cs])
                pts.append(pt)

            for i, w in enumerate(JOB_COLS):
                mask = mpool.tile([P, w], f8, tag="m", padded_shape=[P, max_w])
                nc.scalar.activation(
                    out=mask[:], in_=tts[i][:], func=AF.Sign,
                    accum_out=acc[:, NJ + i : NJ + i + 1],
                )
                dsq = qpool.tile([P, w], f8, tag="q", padded_shape=[P, max_w])
                nc.vector._custom_dve(
                    msd_op,
                    out=dsq[:], in0=tts[i][:], in1=pts[i][:],
                    s0=0.0, s1=0.0,
                    accum_out=acc[:, i : i + 1],
                )

            nc.sync.dma_start(out=out_d[:], in_=acc[:])

    nc.compile()
    _CACHED_NC = nc
    return nc


def _pack_cores(t2: np.ndarray, p2: np.ndarray):
    """Round both images to fp8 (RTNE) and lay each core's row block out as
    [128, 16384] (any bijective relayout is valid: the device only reduces)."""
    t8 = t2.astype(HOST_DT)
    p8 = p2.astype(HOST_DT)
    in_maps = []
    for c in range(N_CORES):
        rs = slice(c * ROWS_PER_CORE, (c + 1) * ROWS_PER_CORE)
        in_maps.append({
            "t": np.ascontiguousarray(t8[rs]).reshape(P, COLS),
            "p": np.ascontiguousarray(p8[rs]).reshape(P, COLS),
        })
    return in_maps


def _run_device(t2: np.ndarray, p2: np.ndarray, trace: bool = False):
    from concourse.bass_utils import run_bass_kernel_spmd

    nc = _build_program()
    in_maps = _pack_cores(t2, p2)
    return run_bass_kernel_spmd(nc, in_maps, list(range(N_CORES)), trace=trace)


def kernel(pred: np.ndarray, target: np.ndarray) -> np.ndarray:
    p2 = np.ascontiguousarray(np.asarray(pred, dtype=np.float32).reshape(H, W))
    t2 = np.ascontiguousarray(np.asarray(target, dtype=np.float32).reshape(H, W))

    results = _run_device(t2, p2).results

    S = 0.0
    C = 0.0
    for c in range(N_CORES):
        o = results[c]["o"].astype(np.float64)
        S += float(o[:, :NJ].sum())
        C += float(o[:, NJ:].sum())
    mse = S / max(C, 1.0)

    corner = 0.0
    for wi, i in zip(_BORDER_W, _BORDER_IDX):
        for wj, j in zip(_BORDER_W, _BORDER_IDX):
            corner += wi * wj * (float(t2[i, j]) - float(p2[i, j]))
    mean_g = corner / (H * W)

    return np.asarray(0.2 * mean_g + 0.8 * mse, dtype=np.float32)
